# revision 1
# baseline (speedup 1.0000x reference)
"""Trainium2 Bass kernel for nn_Decoder_35527969472565.

Contract: kernel(**inputs) takes the FULL unsharded inputs (as produced by
setup_inputs()) and returns the FULL [32, 400, 80] float32 output.

The attention softmax is invariant to the per-(b,t) additive term
(tanh(h1) @ wa_h + ba is constant along the softmax axis), so the attention
context is step-independent and precomputable; the model reduces to two
LSTM recurrences plus batched GEMMs. The device program is single-core
(replicated across the 8 NeuronCores via SPMD); the serial recurrence
dominates and does not benefit from sharding without per-step collectives,
whose latency floor exceeds the per-step compute on this platform.
"""

import numpy as np

import concourse.bacc as bacc
import concourse.mybir as mybir
import concourse.tile as tile

F32 = mybir.dt.float32
AF = mybir.ActivationFunctionType

B = 32
S = 200
E2 = 512
H = 1024
M = 80
NG = 8          # gate groups (512 cols each)
GW = 512        # group width
KC = 8          # K chunks of 128 over H
G4 = 4 * H


def gate_perm():
    P = []
    for n in range(NG):
        P += list(range(0 * H + n * 128, 0 * H + (n + 1) * 128))   # i
        P += list(range(1 * H + n * 128, 1 * H + (n + 1) * 128))   # f
        P += list(range(3 * H + n * 128, 3 * H + (n + 1) * 128))   # o
        P += list(range(2 * H + n * 128, 2 * H + (n + 1) * 128))   # g
    return np.array(P)


def prep_inputs(inp, T):
    P = gate_perm()
    f32 = np.float32
    Wih0 = np.asarray(inp["Wih0"], f32)[P]
    Whh0 = np.asarray(inp["Whh0"], f32)[P]
    Wih1 = np.asarray(inp["Wih1"], f32)[P]
    Whh1 = np.asarray(inp["Whh1"], f32)[P]
    b0 = (np.asarray(inp["bih0"], f32) + np.asarray(inp["bhh0"], f32))[P]
    b1 = (np.asarray(inp["bih1"], f32) + np.asarray(inp["bhh1"], f32))[P]
    enc = np.asarray(inp["encoder_outputs"], f32)
    x = np.asarray(inp["audio_targets"], f32)[:, :T, :]

    d = {}
    d["WhhT0"] = np.ascontiguousarray(Whh0.T)
    d["WhhT1"] = np.ascontiguousarray(Whh1.T)
    d["WihT1"] = np.ascontiguousarray(Wih1.T)
    d["WihxT"] = np.ascontiguousarray(Wih0[:, :M].T)
    d["WiheT"] = np.ascontiguousarray(Wih0[:, M:].T)
    d["WfcT"] = np.ascontiguousarray(np.asarray(inp["Wfc"], f32).T)
    d["b0row"] = b0.reshape(1, G4)
    d["b1row"] = b1.reshape(1, G4)
    d["bfcrow"] = np.asarray(inp["bfc"], f32).reshape(1, M)
    d["enc"] = np.ascontiguousarray(enc)
    d["waeRep"] = np.tile(np.asarray(inp["Wa"], np.float32)[0:1, H:], (128, 1))
    d["w0col"] = np.full((S, 1), 1.0 / S, f32)
    d["xT"] = np.ascontiguousarray(x.transpose(2, 1, 0).reshape(M, T * B))
    d["i32"] = np.eye(B, dtype=f32)
    d["ones128"] = np.ones((1, 128), f32)
    return d


def build(T=400):
    nc = bacc.Bacc()
    names = ["WhhT0", "WhhT1", "WihT1", "WihxT", "WiheT", "WfcT", "b0row",
             "b1row", "bfcrow", "enc", "waeRep", "w0col", "xT", "i32",
             "ones128"]
    shapes = {"WhhT0": (H, G4), "WhhT1": (H, G4), "WihT1": (H, G4),
              "WihxT": (M, G4), "WiheT": (E2, G4), "WfcT": (H, M),
              "b0row": (1, G4), "b1row": (1, G4), "bfcrow": (1, M),
              "enc": (B, S, E2), "waeRep": (128, E2),
              "w0col": (S, 1), "xT": (M, T * B), "i32": (B, B),
              "ones128": (1, 128)}
    t_in = {n: nc.dram_tensor(n, list(shapes[n]), F32, kind="ExternalInput")
            for n in names}
    out = nc.dram_tensor("out", [B, T, M], F32, kind="ExternalOutput")

    H0T = nc.dram_tensor("H0T", [T, 128, 256], F32)
    H1T = nc.dram_tensor("H1T", [T, 128, 256], F32)
    Q1 = nc.dram_tensor("Q1", [T * B, G4], F32)
    C0D = nc.dram_tensor("C0D", [2, B, G4], F32)   # [0]=run, [1]=init

    with tile.TileContext(nc) as tc:
        with (
            tc.tile_pool(name="wp", bufs=1) as wp,
            tc.tile_pool(name="sb", bufs=2) as sb,
            tc.tile_pool(name="sb3", bufs=3) as sb3,
            tc.tile_pool(name="psg", bufs=3, space="PSUM") as psg,
            tc.tile_pool(name="psb", bufs=1, space="PSUM") as psb,
        ):
            # ---------- resident small tensors ----------
            wihx = wp.tile([M, G4], F32, tag="wihx")
            nc.sync.dma_start(wihx[:], t_in["WihxT"][:])
            bfcr = wp.tile([1, M], F32, tag="bfcr")
            nc.sync.dma_start(bfcr[:], t_in["bfcrow"][:])
            i32t = wp.tile([B, B], F32, tag="i32t")
            nc.sync.dma_start(i32t[:], t_in["i32"][:])
            ones128t = wp.tile([1, 128], F32, tag="ones128t")
            nc.sync.dma_start(ones128t[:], t_in["ones128"][:])
            waer = wp.tile([128, E2], F32, tag="waer")
            nc.sync.dma_start(waer[:], t_in["waeRep"][:])
            w0a = wp.tile([128, 1], F32, tag="w0a")
            nc.sync.dma_start(w0a[:], t_in["w0col"][0:128, :])
            w0c = wp.tile([72, 1], F32, tag="w0c")
            nc.sync.dma_start(w0c[:], t_in["w0col"][128:200, :])

            # ---------- attention: scores via DVE reduce ----------
            scT_a = wp.tile([128, B], F32, tag="scT_a")
            scT_c = wp.tile([96, B], F32, tag="scT_c")
            nc.vector.memset(scT_c[:], 0.0)
            for b in range(B):
                ea = sb.tile([128, E2], F32, tag="slab", name=f"s_ea{b}")
                nc.sync.dma_start(ea[:], t_in["enc"][b, 0:128, :])
                ec = sb.tile([72, E2], F32, tag="th", name=f"s_ec{b}")
                nc.sync.dma_start(ec[:], t_in["enc"][b, 128:200, :])
                tha = sb.tile([128, E2], F32, tag="qsb", name=f"tha{b}")
                nc.scalar.activation(tha[:], ea[:], AF.Tanh)
                thc = sb.tile([72, E2], F32, tag="thc")
                nc.scalar.activation(thc[:], ec[:], AF.Tanh)
                pra = sb.tile([128, E2], F32, tag="slab", name=f"pra{b}")
                nc.vector.tensor_mul(out=pra[:], in0=tha[:], in1=waer[:])
                prc = sb.tile([72, E2], F32, tag="th", name=f"prc{b}")
                nc.vector.tensor_mul(out=prc[:], in0=thc[:], in1=waer[0:72, :])
                nc.vector.reduce_sum(scT_a[:, b:b + 1], pra[:],
                                     axis=mybir.AxisListType.X)
                nc.vector.reduce_sum(scT_c[0:72, b:b + 1], prc[:],
                                     axis=mybir.AxisListType.X)
            score = wp.tile([B, 224], F32, tag="score")
            for j in range(4):
                nc.vector.transpose(score[:, 32 * j:32 * (j + 1)],
                                    scT_a[32 * j:32 * (j + 1), :])
            for j in range(3):
                nc.vector.transpose(score[:, 128 + 32 * j:160 + 32 * j],
                                    scT_c[32 * j:32 * (j + 1), :])

            mx = sb.tile([B, 1], F32, tag="mx")
            nc.vector.reduce_max(mx[:], score[:, 0:S], axis=mybir.AxisListType.X)
            nmx = sb.tile([B, 1], F32, tag="nmx")
            nc.vector.tensor_scalar_mul(nmx[:], mx[:], -1.0)
            ew = wp.tile([B, 224], F32, tag="ew")
            nc.vector.memset(ew[:], 0.0)
            nc.scalar.activation(ew[:, 0:S], score[:, 0:S], AF.Exp, bias=nmx[:])
            sm = sb.tile([B, 1], F32, tag="sm")
            nc.vector.reduce_sum(sm[:], ew[:, 0:S], axis=mybir.AxisListType.X)
            rs = sb.tile([B, 1], F32, tag="rs")
            nc.vector.reciprocal(rs[:], sm[:])
            wgt = wp.tile([B, 224], F32, tag="wgt")
            nc.vector.tensor_scalar_mul(wgt[:], ew[:], rs[:])
            wT_a = wp.tile([128, B], F32, tag="wT_a")
            wT_c = wp.tile([96, B], F32, tag="wT_c")
            for j in range(4):
                nc.vector.transpose(wT_a[32 * j:32 * (j + 1), :],
                                    wgt[:, 32 * j:32 * (j + 1)])
            for j in range(3):
                nc.vector.transpose(wT_c[32 * j:32 * (j + 1), :],
                                    wgt[:, 128 + 32 * j:128 + 32 * (j + 1)])

            def ctx_tiles(rcol_a, rcol_c, tagbase):
                cps = [psb.tile([128, B], F32, tag=f"b{hs}", name=f"ctxps_{tagbase}_{hs}")
                       for hs in range(4)]
                for b in range(B):
                    ea = sb.tile([128, E2], F32, tag="slab")
                    nc.sync.dma_start(ea[:], t_in["enc"][b, 0:128, :])
                    ec = sb.tile([72, E2], F32, tag="th")
                    nc.sync.dma_start(ec[:], t_in["enc"][b, 128:200, :])
                    for hs in range(4):
                        nc.tensor.matmul(cps[hs][:, b:b + 1],
                                         ea[:, 128 * hs:128 * (hs + 1)],
                                         rcol_a(b), start=True, stop=False)
                        nc.tensor.matmul(cps[hs][:, b:b + 1],
                                         ec[:, 128 * hs:128 * (hs + 1)],
                                         rcol_c(b), start=False, stop=True)
                outs = []
                for hs in range(4):
                    ct = wp.tile([128, B], F32, tag=f"{tagbase}{hs}")
                    nc.scalar.activation(ct[:], cps[hs][:], AF.Copy)
                    outs.append(ct)
                return outs

            ctxT = ctx_tiles(lambda b: wT_a[:, b:b + 1],
                             lambda b: wT_c[0:72, b:b + 1], "cT")
            ctx0T = ctx_tiles(lambda b: w0a[:], lambda b: w0c[:], "c0T")

            # WiheT shares the big-weight slot (loaded before Whh0T)
            wihe = wp.tile([128, 4 * G4], F32, tag="bigW")
            nc.sync.dma_start(wihe[:],
                              t_in["WiheT"].ap().rearrange("(c p) n -> p c n", p=128))

            for idx, ctx_t in ((0, ctxT), (1, ctx0T)):
                for n in range(NG):
                    cps = psg.tile([B, GW], F32, tag="g")
                    nc.tensor.matmul(cps[:], ctx_t[0][:],
                                     wihe[:, 0 * G4 + GW * n:0 * G4 + GW * (n + 1)],
                                     start=True, stop=False)
                    for hs in range(1, 4):
                        nc.tensor.matmul(
                            cps[:], ctx_t[hs][:],
                            wihe[:, G4 * hs + GW * n:G4 * hs + GW * (n + 1)],
                            start=False, stop=False)
                    # + b0 broadcast to 32 rows: ones128t[:, 0:1].T? use i32-style:
                    # lhsT = ones [1,B] slice of ones128t -> out rows 0..31
                    b0sl = sb3.tile([1, GW], F32, tag="brow", name=f"b0sl_{idx}_{n}")
                    nc.sync.dma_start(b0sl[:], t_in["b0row"][:, GW * n:GW * (n + 1)])
                    nc.tensor.matmul(cps[:], ones128t[:, 0:B], b0sl[:],
                                     start=False, stop=True)
                    csb = sb3.tile([B, GW], F32, tag="padd")
                    nc.scalar.activation(csb[:], cps[:], AF.Copy)
                    nc.sync.dma_start(C0D[idx, :, GW * n:GW * (n + 1)], csb[:])

            # ---------- shared LSTM cell elementwise ----------
            def cell_elem(g_ps, cprev, hT_next, n):
                sg = sb3.tile([B, 384], F32, tag="sg")
                nc.scalar.activation(sg[:], g_ps[:, 0:384], AF.Sigmoid)
                tg = sb3.tile([B, 128], F32, tag="tg")
                nc.scalar.activation(tg[:], g_ps[:, 384:512], AF.Tanh)
                t2 = sb.tile([B, 128], F32, tag="t2")
                nc.vector.tensor_mul(out=t2[:], in0=sg[:, 0:128], in1=tg[:])
                cnew = sb.tile([B, 128], F32, tag=f"c{n}")
                if cprev is None:
                    nc.vector.tensor_copy(cnew[:], t2[:])
                else:
                    t1 = sb.tile([B, 128], F32, tag="t1")
                    nc.vector.tensor_mul(out=t1[:], in0=sg[:, 128:256], in1=cprev[:])
                    nc.vector.tensor_add(out=cnew[:], in0=t1[:], in1=t2[:])
                tc2 = sb.tile([B, 128], F32, tag="tc2")
                nc.scalar.activation(tc2[:], cnew[:], AF.Tanh)
                hn = sb.tile([B, 128], F32, tag="hn")
                nc.vector.tensor_mul(out=hn[:], in0=sg[:, 256:384], in1=tc2[:])
                for j in range(4):
                    nc.vector.transpose(
                        hT_next[32 * j:32 * (j + 1), 32 * n:32 * (n + 1)],
                        hn[:, 32 * j:32 * (j + 1)])
                return cnew

            # ---------- phase A: layer-0 recurrence ----------
            big = wp.tile([128, KC * G4], F32, tag="bigW")
            nc.sync.dma_start(big[:],
                              t_in["WhhT0"].ap().rearrange("(c p) n -> p c n", p=128))
            hT = None
            cst = [None] * NG
            for t in range(T):
                xsl = sb3.tile([M, B], F32, tag="xsl")
                nc.sync.dma_start(xsl[:], t_in["xT"][:, B * t:B * (t + 1)])
                hT_next = sb.tile([128, 256], F32, tag="hTn")
                for n in range(NG):
                    padd = sb3.tile([B, GW], F32, tag="padd")
                    nc.sync.dma_start(padd[:],
                                      C0D[1 if t == 0 else 0, :, GW * n:GW * (n + 1)])
                    g_ps = psg.tile([B, GW], F32, tag="g")
                    nc.tensor.matmul(g_ps[:], xsl[:], wihx[:, GW * n:GW * (n + 1)],
                                     start=True, stop=False)
                    nc.tensor.matmul(g_ps[:], i32t[:], padd[:],
                                     start=False, stop=(hT is None))
                    if hT is not None:
                        for c in range(KC):
                            nc.tensor.matmul(
                                g_ps[:], hT[:, 32 * c:32 * (c + 1)],
                                big[:, G4 * c + GW * n:G4 * c + GW * (n + 1)],
                                start=False, stop=(c == KC - 1))
                    cst[n] = cell_elem(g_ps, cst[n], hT_next, n)
                nc.scalar.dma_start(H0T[t], hT_next[:])
                hT = hT_next

            # ---------- mid: Q1 = H0 @ Wih1T + b1 ----------
            big = wp.tile([128, KC * G4], F32, tag="bigW")
            nc.sync.dma_start(big[:],
                              t_in["WihT1"].ap().rearrange("(c p) n -> p c n", p=128))
            assert T % 4 == 0
            for m in range(T // 4):
                slab = sb.tile([128, 1024], F32, tag="slab", name=f"mslab{m}")
                nc.sync.dma_start(
                    slab[:],
                    H0T.ap()[4 * m:4 * (m + 1)].rearrange(
                        "t p (c b) -> p c t b", c=8))
                for n in range(NG):
                    b1sl = sb3.tile([1, GW], F32, tag="brow", name=f"b1sl_{m}_{n}")
                    nc.sync.dma_start(b1sl[:], t_in["b1row"][:, GW * n:GW * (n + 1)])
                    qps = psg.tile([128, GW], F32, tag="g")
                    nc.tensor.matmul(qps[:], ones128t[:], b1sl[:],
                                     start=True, stop=False)
                    for c in range(KC):
                        nc.tensor.matmul(
                            qps[:], slab[:, 128 * c:128 * (c + 1)],
                            big[:, G4 * c + GW * n:G4 * c + GW * (n + 1)],
                            start=False, stop=(c == KC - 1))
                    qsb = sb.tile([128, GW], F32, tag="qsb")
                    nc.scalar.activation(qsb[:], qps[:], AF.Copy)
                    nc.sync.dma_start(
                        Q1[128 * m:128 * (m + 1), GW * n:GW * (n + 1)], qsb[:])

            # ---------- phase B: layer-1 recurrence ----------
            big = wp.tile([128, KC * G4], F32, tag="bigW")
            nc.sync.dma_start(big[:],
                              t_in["WhhT1"].ap().rearrange("(c p) n -> p c n", p=128))
            hT = None
            cst = [None] * NG
            for t in range(T):
                hT_next = sb.tile([128, 256], F32, tag="hTn")
                for n in range(NG):
                    padd = sb3.tile([B, GW], F32, tag="padd")
                    nc.sync.dma_start(padd[:],
                                      Q1[B * t:B * (t + 1), GW * n:GW * (n + 1)])
                    g_ps = psg.tile([B, GW], F32, tag="g")
                    nc.tensor.matmul(g_ps[:], i32t[:], padd[:],
                                     start=True, stop=(hT is None))
                    if hT is not None:
                        for c in range(KC):
                            nc.tensor.matmul(
                                g_ps[:], hT[:, 32 * c:32 * (c + 1)],
                                big[:, G4 * c + GW * n:G4 * c + GW * (n + 1)],
                                start=False, stop=(c == KC - 1))
                    cst[n] = cell_elem(g_ps, cst[n], hT_next, n)
                nc.scalar.dma_start(H1T[t], hT_next[:])
                hT = hT_next

            # ---------- fc ----------
            wfc = wp.tile([128, 8 * M], F32, tag="wihx")  # reuse slot, fits
            nc.sync.dma_start(wfc[:],
                              t_in["WfcT"].ap().rearrange("(c p) m -> p c m", p=128))
            for m in range(T // 4):
                slab = sb.tile([128, 1024], F32, tag="slab", name=f"fslab{m}")
                nc.sync.dma_start(
                    slab[:],
                    H1T.ap()[4 * m:4 * (m + 1)].rearrange(
                        "t p (c b) -> p c t b", c=8))
                pps = psg.tile([128, M], F32, tag="g")
                nc.tensor.matmul(pps[:], ones128t[:], bfcr[:], start=True, stop=False)
                for c in range(KC):
                    nc.tensor.matmul(pps[:], slab[:, 128 * c:128 * (c + 1)],
                                     wfc[:, M * c:M * (c + 1)],
                                     start=False, stop=(c == KC - 1))
                pout = sb.tile([128, M], F32, tag="qsb")
                nc.scalar.activation(pout[:], pps[:], AF.Copy)
                for tt in range(4):
                    nc.sync.dma_start(out[:, 4 * m + tt, :],
                                      pout[32 * tt:32 * (tt + 1), :])
    nc.finalize()
    return nc, names


_CACHE = {}


def kernel(**inputs):
    import numpy as np
    from concourse.bass_utils import run_bass_kernel_spmd

    T = int(np.asarray(inputs["audio_targets"]).shape[1])
    if T not in _CACHE:
        _CACHE[T] = build(T)
    nc, _names = _CACHE[T]
    d = prep_inputs(inputs, T)
    n_cores = 8
    in_maps = [dict(d) for _ in range(n_cores)]
    res = run_bass_kernel_spmd(nc, in_maps, list(range(n_cores)))
    return np.asarray(res.results[0]["out"], dtype=np.float32)



# revision 11
# speedup vs baseline: 2.5751x; 2.5751x over previous
"""Trainium2 Bass kernel for nn_Decoder_35527969472565.

Contract: kernel(**inputs) takes the FULL unsharded inputs (as produced by
setup_inputs()) and returns the FULL [32, 400, 80] float32 output.

Attention softmax is shift-invariant along S, so the (b,t)-additive term
drops out and the attention context is step-independent: the model reduces
to two LSTM recurrences plus batched GEMMs.

Layout: gates live in PSUM banks of [128, 512] where the partition axis
packs (group j, batch b) for 4 gate-groups and the free axis is the
[i|f|o|g] interleave of one group (gate_perm). The recurrent GEMM uses
4-way tile_position column tiling with bf16 operands (measured ~70 ns per
[K=128,M=32,N=512] matmul vs ~430 ns for plain fp32), and h is kept
transposed via PE-transposes so each step's stationary operands are ready.
Weights are bf16 (PSUM accumulation stays fp32); the per-step context/bias
injection stays fp32 via identity matmuls.

Single device program replicated across the 8 NeuronCores.
"""

import numpy as np

import concourse.bacc as bacc
import concourse.mybir as mybir
import concourse.tile as tile

F32 = mybir.dt.float32
BF16 = mybir.dt.bfloat16
AF = mybir.ActivationFunctionType

B = 32
S = 200
E2 = 512
H = 1024
M = 80
G4 = 4 * H


def gate_perm():
    P = []
    for n in range(8):
        P += list(range(0 * H + n * 128, 0 * H + (n + 1) * 128))   # i
        P += list(range(1 * H + n * 128, 1 * H + (n + 1) * 128))   # f
        P += list(range(3 * H + n * 128, 3 * H + (n + 1) * 128))   # o
        P += list(range(2 * H + n * 128, 2 * H + (n + 1) * 128))   # g
    return np.array(P)


def _chunks(WT, n):
    """[K, G] -> [n, 128, G] k-chunk major."""
    K = WT.shape[0]
    assert K == 128 * n
    return np.ascontiguousarray(WT.reshape(n, 128, -1))


def prep_inputs(inp, T):
    import ml_dtypes
    bf = ml_dtypes.bfloat16
    P = gate_perm()
    f32 = np.float32
    Wih0 = np.asarray(inp["Wih0"], f32)[P]
    Whh0 = np.asarray(inp["Whh0"], f32)[P]
    Wih1 = np.asarray(inp["Wih1"], f32)[P]
    Whh1 = np.asarray(inp["Whh1"], f32)[P]
    b0 = (np.asarray(inp["bih0"], f32) + np.asarray(inp["bhh0"], f32))[P]
    b1 = (np.asarray(inp["bih1"], f32) + np.asarray(inp["bhh1"], f32))[P]
    enc = np.asarray(inp["encoder_outputs"], f32)
    x = np.asarray(inp["audio_targets"], f32)[:, :T, :]

    d = {}
    d["Whh0bk"] = _chunks(Whh0.T, 8).astype(bf)
    d["Whh1bk"] = _chunks(Whh1.T, 8).astype(bf)
    d["Wih1bk"] = _chunks(Wih1.T, 8).astype(bf)
    d["Wihxbk"] = np.ascontiguousarray(Wih0[:, :M].T).astype(bf)   # [80, 4096]
    d["Wihebk"] = _chunks(np.ascontiguousarray(Wih0[:, M:].T), 4).astype(bf)
    d["Wfcbk"] = _chunks(
        np.ascontiguousarray(np.asarray(inp["Wfc"], f32).T), 8).astype(bf)
    # banked b0: [128=(j,b), 1024=(r,c128x4)]  group g=4r+j cols c
    b0bk = np.zeros((128, 1024), f32)
    for r in range(2):
        for j in range(4):
            g = 4 * r + j
            b0bk[32 * j:32 * (j + 1), 512 * r:512 * (r + 1)] = \
                b0[512 * g:512 * (g + 1)][None, :]
    d["b0bk"] = b0bk
    d["b1row"] = b1.reshape(1, G4)
    d["bfcrow"] = np.asarray(inp["bfc"], f32).reshape(1, M)
    d["enc"] = np.ascontiguousarray(enc)
    d["waeRep"] = np.tile(np.asarray(inp["Wa"], f32)[0:1, H:], (128, 1))
    d["w0col"] = np.full((S, 1), 1.0 / S, f32)
    d["xT"] = np.ascontiguousarray(
        x.transpose(2, 1, 0).reshape(M, T * B)).astype(bf)
    d["i128f"] = np.eye(128, dtype=f32)
    d["i128b"] = np.eye(128, dtype=f32).astype(bf)
    d["ones1x128"] = np.ones((1, 128), f32)
    return d


def build(T=400):
    nc = bacc.Bacc()
    shapes = {
        "Whh0bk": ((8, 128, G4), BF16), "Whh1bk": ((8, 128, G4), BF16),
        "Wih1bk": ((8, 128, G4), BF16), "Wihxbk": ((M, G4), BF16),
        "Wihebk": ((4, 128, G4), BF16), "Wfcbk": ((8, 128, M), BF16),
        "b0bk": ((128, 1024), F32), "b1row": ((1, G4), F32),
        "bfcrow": ((1, M), F32), "enc": ((B, S, E2), F32),
        "waeRep": ((128, E2), F32), "w0col": ((S, 1), F32),
        "xT": ((M, T * B), BF16), "i128f": ((128, 128), F32),
        "i128b": ((128, 128), BF16), "ones1x128": ((1, 128), F32),
    }
    t_in = {n: nc.dram_tensor(n, list(sh), dt, kind="ExternalInput")
            for n, (sh, dt) in shapes.items()}
    out = nc.dram_tensor("out", [B, T, M], F32, kind="ExternalOutput")

    HT0 = nc.dram_tensor("HT0", [T, 128, 256], BF16)   # hT0 per step
    Q1D = nc.dram_tensor("Q1D", [T, 128, 1024], BF16)  # banked Wih1@h0 + b1

    with tile.TileContext(nc) as tc:
        with (
            tc.tile_pool(name="wp", bufs=1) as wp,
            tc.tile_pool(name="sb", bufs=2) as sb,
            tc.tile_pool(name="sb3", bufs=3) as sb3,
            tc.tile_pool(name="psg", bufs=2, space="PSUM") as psg,
            tc.tile_pool(name="pst", bufs=2, space="PSUM") as pst,
            tc.tile_pool(name="psf", bufs=2, space="PSUM") as psf,
        ):
            # ---------- small resident tensors ----------
            i128f = wp.tile([128, 128], F32, tag="i128f")
            nc.sync.dma_start(i128f[:], t_in["i128f"][:])
            i128b = wp.tile([128, 128], BF16, tag="i128b")
            nc.sync.dma_start(i128b[:], t_in["i128b"][:])
            ones128 = wp.tile([1, 128], F32, tag="ones128")
            nc.sync.dma_start(ones128[:], t_in["ones1x128"][:])
            b0bk = wp.tile([128, 1024], F32, tag="b0bk")
            nc.sync.dma_start(b0bk[:], t_in["b0bk"][:])
            bfcr = wp.tile([1, M], F32, tag="bfcr")
            nc.sync.dma_start(bfcr[:], t_in["bfcrow"][:])
            b1r = wp.tile([1, G4], F32, tag="b1r")
            nc.sync.dma_start(b1r[:], t_in["b1row"][:])
            wihx = wp.tile([M, G4], BF16, tag="wihx")
            nc.sync.dma_start(wihx[:], t_in["Wihxbk"][:])
            xT = wp.tile([M, T * B], BF16, tag="xT")
            nc.sync.dma_start(xT[:], t_in["xT"][:])
            waer = wp.tile([128, E2], F32, tag="waer")
            nc.sync.dma_start(waer[:], t_in["waeRep"][:])
            w0a = wp.tile([128, 1], F32, tag="w0a")
            nc.sync.dma_start(w0a[:], t_in["w0col"][0:128, :])
            w0c = wp.tile([72, 1], F32, tag="w0c")
            nc.sync.dma_start(w0c[:], t_in["w0col"][128:200, :])

            # ---------- attention scores (softmax weights, step-indep) ----
            scT_a = wp.tile([128, B], F32, tag="scT_a")
            scT_c = wp.tile([96, B], F32, tag="scT_c")
            nc.vector.memset(scT_c[:], 0.0)
            for b in range(B):
                ea = sb.tile([128, E2], F32, tag="slab", name=f"s_ea{b}")
                nc.sync.dma_start(ea[:], t_in["enc"][b, 0:128, :])
                ec = sb.tile([72, E2], F32, tag="th", name=f"s_ec{b}")
                nc.sync.dma_start(ec[:], t_in["enc"][b, 128:200, :])
                tha = sb.tile([128, E2], F32, tag="qsb", name=f"tha{b}")
                nc.scalar.activation(tha[:], ea[:], AF.Tanh)
                thc = sb.tile([72, E2], F32, tag="thc")
                nc.scalar.activation(thc[:], ec[:], AF.Tanh)
                pra = sb.tile([128, E2], F32, tag="slab", name=f"pra{b}")
                nc.vector.tensor_mul(out=pra[:], in0=tha[:], in1=waer[:])
                prc = sb.tile([72, E2], F32, tag="th", name=f"prc{b}")
                nc.vector.tensor_mul(out=prc[:], in0=thc[:], in1=waer[0:72, :])
                nc.vector.reduce_sum(scT_a[:, b:b + 1], pra[:],
                                     axis=mybir.AxisListType.X)
                nc.vector.reduce_sum(scT_c[0:72, b:b + 1], prc[:],
                                     axis=mybir.AxisListType.X)
            score = wp.tile([B, 224], F32, tag="score")
            for j in range(4):
                nc.vector.transpose(score[:, 32 * j:32 * (j + 1)],
                                    scT_a[32 * j:32 * (j + 1), :])
            for j in range(3):
                nc.vector.transpose(score[:, 128 + 32 * j:160 + 32 * j],
                                    scT_c[32 * j:32 * (j + 1), :])
            mx = sb.tile([B, 1], F32, tag="mx")
            nc.vector.reduce_max(mx[:], score[:, 0:S], axis=mybir.AxisListType.X)
            nmx = sb.tile([B, 1], F32, tag="nmx")
            nc.vector.tensor_scalar_mul(nmx[:], mx[:], -1.0)
            ew = wp.tile([B, 224], F32, tag="ew")
            nc.vector.memset(ew[:], 0.0)
            nc.scalar.activation(ew[:, 0:S], score[:, 0:S], AF.Exp, bias=nmx[:])
            sm = sb.tile([B, 1], F32, tag="sm")
            nc.vector.reduce_sum(sm[:], ew[:, 0:S], axis=mybir.AxisListType.X)
            rs = sb.tile([B, 1], F32, tag="rs")
            nc.vector.reciprocal(rs[:], sm[:])
            wgt = wp.tile([B, 224], F32, tag="wgt")
            nc.vector.tensor_scalar_mul(wgt[:], ew[:], rs[:])
            wT_a = wp.tile([128, B], F32, tag="wT_a")
            wT_c = wp.tile([96, B], F32, tag="wT_c")
            for j in range(4):
                nc.vector.transpose(wT_a[32 * j:32 * (j + 1), :],
                                    wgt[:, 32 * j:32 * (j + 1)])
            for j in range(3):
                nc.vector.transpose(wT_c[32 * j:32 * (j + 1), :],
                                    wgt[:, 128 + 32 * j:128 + 32 * (j + 1)])

            # ---------- ctxT tiles: [128 e-chunk, B] x4, fp32 -> bf16 -----
            def ctx_tiles(rcol_a, rcol_c, tagbase):
                cpsall = psf.tile([128, 4 * B], F32, tag="fc",
                                  name=f"ctxps_{tagbase}")
                for b in range(B):
                    ea = sb.tile([128, E2], F32, tag="slab", name=f"c_ea{tagbase}{b}")
                    nc.sync.dma_start(ea[:], t_in["enc"][b, 0:128, :])
                    ec = sb.tile([72, E2], F32, tag="th", name=f"c_ec{tagbase}{b}")
                    nc.sync.dma_start(ec[:], t_in["enc"][b, 128:200, :])
                    for hs in range(4):
                        nc.tensor.matmul(cpsall[:, B * hs + b:B * hs + b + 1],
                                         ea[:, 128 * hs:128 * (hs + 1)],
                                         rcol_a(b), start=True, stop=False)
                        nc.tensor.matmul(cpsall[:, B * hs + b:B * hs + b + 1],
                                         ec[:, 128 * hs:128 * (hs + 1)],
                                         rcol_c(b), start=False, stop=True)
                outs = []
                for hs in range(4):
                    ct = wp.tile([128, B], BF16, tag=f"{tagbase}{hs}")
                    nc.scalar.activation(ct[:], cpsall[:, B * hs:B * (hs + 1)],
                                         AF.Copy)
                    outs.append(ct)
                return outs

            ctxT = ctx_tiles(lambda b: wT_a[:, b:b + 1],
                             lambda b: wT_c[0:72, b:b + 1], "cT")
            ctx0T = ctx_tiles(lambda b: w0a[:], lambda b: w0c[:], "c0T")

            # ---------- cb0 / cb0i: banked ctx@WiheT + b0  [128,1024] f32 --
            wihe = wp.tile([128, 4 * G4], BF16, tag="bigW")
            nc.sync.dma_start(wihe[:],
                              t_in["Wihebk"].ap().rearrange("k p n -> p k n"))
            cbs = []
            for idx, ct in ((0, ctxT), (1, ctx0T)):
                cps = psg.tile([128, 1024], F32, tag="g", name=f"cbps{idx}")
                for r in range(2):
                    nc.tensor.matmul(cps[:, 512 * r:512 * (r + 1)], i128f[:],
                                     b0bk[:, 512 * r:512 * (r + 1)],
                                     start=True, stop=False)
                    for k in range(4):
                        for j in range(4):
                            g = 4 * r + j
                            nc.tensor.matmul(
                                cps[32 * j:32 * (j + 1), 512 * r:512 * (r + 1)],
                                ct[k][:],
                                wihe[:, G4 * k + 512 * g:G4 * k + 512 * (g + 1)],
                                start=False, stop=(k == 3),
                                tile_position=(0, 32 * j),
                                skip_group_check=True)
                cb = wp.tile([128, 1024], F32, tag=f"cb{idx}")
                nc.scalar.activation(cb[:], cps[:], AF.Copy)
                cbs.append(cb)
            cb0, cb0i = cbs

            # ---------- shared per-step cell ----------
            def step_cell(t, gps, c_st, first, hT_tag):
                """EW on gate psum [128,1024] -> new hT tile [128,256] bf16."""
                hbank = sb.tile([128, 256], BF16, tag="hb", name=f"hb_{hT_tag}_{t}")
                for r in range(2):
                    sg = sb3.tile([128, 384], F32, tag="sg")
                    nc.scalar.activation(sg[:], gps[:, 512 * r:512 * r + 384],
                                         AF.Sigmoid)
                    tg = sb3.tile([128, 128], F32, tag="tg")
                    nc.scalar.activation(tg[:], gps[:, 512 * r + 384:512 * (r + 1)],
                                         AF.Tanh)
                    t2 = sb.tile([128, 128], F32, tag="t2")
                    nc.vector.tensor_mul(out=t2[:], in0=sg[:, 0:128], in1=tg[:])
                    if first:
                        nc.vector.tensor_copy(c_st[:, 128 * r:128 * (r + 1)], t2[:])
                    else:
                        t1 = sb.tile([128, 128], F32, tag="t1")
                        nc.vector.tensor_mul(out=t1[:], in0=sg[:, 128:256],
                                             in1=c_st[:, 128 * r:128 * (r + 1)])
                        nc.vector.tensor_add(out=c_st[:, 128 * r:128 * (r + 1)],
                                             in0=t1[:], in1=t2[:])
                    tc2 = sb.tile([128, 128], F32, tag="tc2")
                    nc.scalar.activation(tc2[:], c_st[:, 128 * r:128 * (r + 1)],
                                         AF.Tanh)
                    nc.vector.tensor_mul(out=hbank[:, 128 * r:128 * (r + 1)],
                                         in0=sg[:, 256:384], in1=tc2[:])
                hTn = sb.tile([128, 256], BF16, tag="hTn", name=f"hT_{hT_tag}_{t}")
                for r in range(2):
                    tps = pst.tile([128, 128], BF16, tag="tp")
                    nc.tensor.transpose(tps[:], hbank[:, 128 * r:128 * (r + 1)],
                                        i128b[:])
                    nc.scalar.activation(hTn[:, 128 * r:128 * (r + 1)], tps[:],
                                         AF.Copy)
                return hTn

            def whh_chains(gps, hT, w, with_stop):
                for r in range(2):
                    for k in range(8):
                        kr, kj = k // 4, k % 4
                        for j in range(4):
                            g = 4 * r + j
                            nc.tensor.matmul(
                                gps[32 * j:32 * (j + 1), 512 * r:512 * (r + 1)],
                                hT[:, 128 * kr + 32 * kj:128 * kr + 32 * kj + 32],
                                w[:, G4 * k + 512 * g:G4 * k + 512 * (g + 1)],
                                start=False,
                                stop=(with_stop and k == 7),
                                tile_position=(0, 32 * j),
                                skip_group_check=True)

            # ---------- phase A: layer-0 recurrence ----------
            whh0 = wp.tile([128, 8 * G4], BF16, tag="bigW")
            nc.sync.dma_start(whh0[:],
                              t_in["Whh0bk"].ap().rearrange("k p n -> p k n"))
            c0 = wp.tile([128, 256], F32, tag="c0st")
            hT = None
            for t in range(T):
                gps = psg.tile([128, 1024], F32, tag="g", name=f"gA{t}")
                cbsrc = cb0i if t == 0 else cb0
                for r in range(2):
                    nc.tensor.matmul(gps[:, 512 * r:512 * (r + 1)], i128f[:],
                                     cbsrc[:, 512 * r:512 * (r + 1)],
                                     start=True, stop=False,
                                     skip_group_check=True)
                    for j in range(4):
                        g = 4 * r + j
                        nc.tensor.matmul(
                            gps[32 * j:32 * (j + 1), 512 * r:512 * (r + 1)],
                            xT[:, B * t:B * (t + 1)],
                            wihx[:, 512 * g:512 * (g + 1)],
                            start=False, stop=(hT is None),
                            tile_position=(0, 32 * j),
                            skip_group_check=True)
                if hT is not None:
                    whh_chains(gps, hT, whh0, True)
                hT = step_cell(t, gps, c0, t == 0, "A")
                nc.scalar.dma_start(HT0[t], hT[:])

            # ---------- mid: Q1 = H0 @ Wih1T + b1 (slab GEMM, M=128) ------
            wih1 = wp.tile([128, 8 * G4], BF16, tag="bigW")
            nc.sync.dma_start(wih1[:],
                              t_in["Wih1bk"].ap().rearrange("k p n -> p k n"))
            assert T % 4 == 0
            for m in range(T // 4):
                hts = sb3.tile([128, 8 * 128], BF16, tag="hts", name=f"hts{m}")
                for kk in range(8):
                    kr, kj = kk // 4, kk % 4
                    nc.sync.dma_start(
                        hts[:, 128 * kk:128 * (kk + 1)],
                        HT0.ap()[4 * m:4 * (m + 1),
                                 :, 128 * kr + 32 * kj:128 * kr + 32 * kj + 32]
                        .rearrange("t p c -> p t c"))
                for gg in range(4):
                    qps = psg.tile([128, 1024], F32, tag="g", name=f"q{m}_{gg}")
                    for half in range(2):
                        n = 2 * gg + half
                        nc.tensor.matmul(
                            qps[:, 512 * half:512 * (half + 1)],
                            ones128[:], b1r[:, 512 * n:512 * (n + 1)],
                            start=True, stop=False)
                        for k in range(8):
                            nc.tensor.matmul(
                                qps[:, 512 * half:512 * (half + 1)],
                                hts[:, 128 * k:128 * (k + 1)],
                                wih1[:, G4 * k + 512 * n:G4 * k + 512 * (n + 1)],
                                start=False, stop=(k == 7))
                    qsb = sb.tile([128, 1024], BF16, tag="qsb", name=f"qs{m}_{gg}")
                    nc.scalar.activation(qsb[:], qps[:], AF.Copy)
                    for half in range(2):
                        n = 2 * gg + half
                        nr, nj = n // 4, n % 4
                        for tt in range(4):
                            nc.sync.dma_start(
                                Q1D[4 * m + tt, 32 * nj:32 * (nj + 1),
                                    512 * nr:512 * (nr + 1)],
                                qsb[32 * tt:32 * (tt + 1),
                                    512 * half:512 * (half + 1)])

            # ---------- phase B: layer-1 recurrence + inline fc ----------
            whh1 = wp.tile([128, 8 * G4], BF16, tag="bigW")
            nc.sync.dma_start(whh1[:],
                              t_in["Whh1bk"].ap().rearrange("k p n -> p k n"))
            wfc = wp.tile([128, 8 * M], BF16, tag="wfc")
            nc.sync.dma_start(wfc[:],
                              t_in["Wfcbk"].ap().rearrange("k p n -> p k n"))
            c1 = wp.tile([128, 256], F32, tag="c1st")
            hT = None
            for t in range(T):
                qsl = sb3.tile([128, 1024], BF16, tag="qsl", name=f"qsl{t}")
                nc.sync.dma_start(qsl[:], Q1D[t])
                gps = psg.tile([128, 1024], F32, tag="g", name=f"gB{t}")
                for r in range(2):
                    nc.tensor.matmul(gps[:, 512 * r:512 * (r + 1)], i128b[:],
                                     qsl[:, 512 * r:512 * (r + 1)],
                                     start=True, stop=(hT is None),
                                     skip_group_check=True)
                if hT is not None:
                    whh_chains(gps, hT, whh1, True)
                hT = step_cell(t, gps, c1, t == 0, "B")
                # fc: pred = h1 @ WfcT + bfc
                pf = psf.tile([B, M], F32, tag="fc", name=f"pf{t}")
                nc.tensor.matmul(pf[:], ones128[:, 0:B], bfcr[:],
                                 start=True, stop=False)
                for kk in range(8):
                    kr, kj = kk // 4, kk % 4
                    nc.tensor.matmul(
                        pf[:],
                        hT[:, 128 * kr + 32 * kj:128 * kr + 32 * kj + 32],
                        wfc[:, M * kk:M * (kk + 1)],
                        start=False, stop=(kk == 7))
                po = sb.tile([B, M], F32, tag="po")
                nc.scalar.activation(po[:], pf[:], AF.Copy)
                nc.sync.dma_start(out[:, t, :], po[:])
    nc.finalize()
    return nc, None


_CACHE = {}


def kernel(**inputs):
    import numpy as np
    from concourse.bass_utils import run_bass_kernel_spmd

    T = int(np.asarray(inputs["audio_targets"]).shape[1])
    if T not in _CACHE:
        _CACHE[T] = build(T)
    nc, _ = _CACHE[T]
    d = prep_inputs(inputs, T)
    n_cores = 8
    in_maps = [dict(d) for _ in range(n_cores)]
    res = run_bass_kernel_spmd(nc, in_maps, list(range(n_cores)))
    return np.asarray(res.results[0]["out"], dtype=np.float32)
